# revision 13
# baseline (speedup 1.0000x reference)
"""Trainium2 Bass kernel for nn_Drifting (dual-softmax contrastive drift).

Computes, for x, y_pos, y_neg all [4096, 512] f32:
    dist_pos = cdist(x, y_pos); dist_neg = cdist(x, y_neg) + eye*1e6
    logit = [-dist_pos, -dist_neg] / 0.05          # [4096, 8192]
    A = sqrt(softmax_row(logit) * softmax_col(logit))
    V = (A_pos * rowsum(A_neg)) @ y_pos - (A_neg * rowsum(A_pos)) @ y_neg

Sharding: rows of x across 8 cores (512 rows each); y replicated. Per-core
layout is TRANSPOSED: scores s[j, i] with j (8192) on partitions (64 tiles
of 128) and local i (512) on the free dim, so column-softmax stats are
free-dim reductions and the output matmul contracts j directly.

Host-side prep (cheap numpy, staged once like the inputs): y^T tiles in
DMA-friendly [t, d, c, j] layout, -2*x^T, |y_j|^2 in [128, 64] p-major
layout, |x_i|^2 split into an f32r hi/lo pair, and a per-core [32]-float
diag-mask row-gain vector (1e6 on the core's own 4 diag tiles).

Pass 1 (flat, 64 j-tiles): d2 via 4 f32r matmuls + a K=2 ones-matmul adding
|x|^2; s = sqrt(d2 + |y|^2[bias]); additive diag mask; column mins; then
e1 = exp(l - cl_j) in groups of 16 (Act sqrt/exp tables cannot coexist; 8
table loads total), with column sums from the activation accumulator.

Math: with K = 10 * min(s_local) (per core) and K0 = -2K, the row-max
factor cancels from the final scale entirely:
  U = e1 * colexp_j,  colexp_j = exp(cl_j - c_j/2 - lnC_j/2 + K)
  V_i = (1/R_i) [SU^neg_i (U@y_pos)_i - SU^pos_i (U@y_neg)_i]
  R_i = sum_j w2_j U[j,i],  w2_j = exp(c_j/2 + lnC_j/2 + K)
so pass 2 per tile is ONE DVE scale (U = e1 * colexp) + 5 f32r matmuls
(4 V-blocks + a fused [ones|w2] stats matmul giving SU and R together).
One small AllGather (64 KB, contiguous 256B runs) shares per-core column
stats (local col mins + e1 col sums); it is the only collective.
"""
import numpy as np

N = 4096
D = 512
NCORES = 8
ROWS = N // NCORES          # 512 local rows (i) per core
J = 2 * N                   # 8192 concat dim
JT = J // 128               # 64 j-tiles
NEG0 = 32                   # first neg tile index
TEMP = 0.05
SC = -1.0 / TEMP            # -20
MASK_VAL = 1e6
G = 16                      # act-table group size (sqrt/exp cannot coexist)

_CACHE = {}


def _build_nc():
    import concourse.bass as bass
    from concourse import bacc
    import concourse.mybir as mybir
    import concourse.tile as tile
    from concourse.masks import make_identity
    from contextlib import ExitStack

    F32 = mybir.dt.float32
    F32R = mybir.dt.float32r
    Exp = mybir.ActivationFunctionType.Exp
    Sqrt = mybir.ActivationFunctionType.Sqrt
    Copy = mybir.ActivationFunctionType.Copy
    Ln = mybir.ActivationFunctionType.Ln
    Alu = mybir.AluOpType
    AX = mybir.AxisListType.X

    nc = bacc.Bacc("TRN2", target_bir_lowering=False, debug=False,
                   num_devices=NCORES)

    ytp = nc.dram_tensor("ytp", [JT, 128, 4, 128], F32R, kind="ExternalInput")
    ypos = nc.dram_tensor("ypos", [N, D], F32R, kind="ExternalInput")
    yneg = nc.dram_tensor("yneg", [N, D], F32R, kind="ExternalInput")
    xm2T = nc.dram_tensor("xm2T", [128, 4, ROWS], F32R, kind="ExternalInput")
    xn2 = nc.dram_tensor("xn2", [2, ROWS], F32R, kind="ExternalInput")
    ynp = nc.dram_tensor("ynp", [128, JT], F32, kind="ExternalInput")
    dsel = nc.dram_tensor("dsel", [32], F32, kind="ExternalInput")
    vout = nc.dram_tensor("vout", [ROWS, D], F32, kind="ExternalOutput")
    import os
    _dbg = bool(int(os.environ.get("KERNEL_DEBUG", "0")))
    _nocc = bool(int(os.environ.get("KERNEL_NO_CC", "0")))
    if _dbg:
        dbg_smin = nc.dram_tensor("dbg_smin", [128, JT], F32, kind="ExternalOutput")
        dbg_sloc = nc.dram_tensor("dbg_sloc", [128, JT], F32, kind="ExternalOutput")
        dbg_colexp = nc.dram_tensor("dbg_colexp", [128, JT], F32, kind="ExternalOutput")
        dbg_w2 = nc.dram_tensor("dbg_w2", [128, JT], F32, kind="ExternalOutput")
        dbg_gmin = nc.dram_tensor("dbg_gmin", [128, JT], F32, kind="ExternalOutput")
        dbg_stat = nc.dram_tensor("dbg_stat", [4, ROWS], F32, kind="ExternalOutput")
        dbg_s0 = nc.dram_tensor("dbg_s0", [128, ROWS], F32, kind="ExternalOutput")
        dbg_e10 = nc.dram_tensor("dbg_e10", [128, ROWS], F32, kind="ExternalOutput")
        dbg_tp0 = nc.dram_tensor("dbg_tp0", [128, ROWS], F32, kind="ExternalOutput")

    ones_row_const = nc.inline_tensor(np.ones((1, 128), np.float32), "ones_row_c")
    ones2_const = nc.inline_tensor(np.ones((2, 128), np.float32), "ones2_c")

    with tile.TileContext(nc) as tc, ExitStack() as top:
        st = top.enter_context(tc.tile_pool(name="st", bufs=1))
        dram = top.enter_context(tc.tile_pool(name="dram", bufs=1, space="DRAM"))
        ps_small = top.enter_context(
            tc.tile_pool(name="ps_small", bufs=2, space="PSUM"))

        # ---------------- static tiles ----------------
        ident = st.tile([128, 128], F32)
        make_identity(nc, ident)
        ones_t = st.tile([1, 128], F32R)       # K=1 row of ones (lhsT)
        nc.sync.dma_start(out=ones_t, in_=ones_row_const.ap().bitcast(F32R))
        ones2 = st.tile([2, 128], F32R)        # K=2 rows of ones (lhsT)
        nc.sync.dma_start(out=ones2, in_=ones2_const.ap().bitcast(F32R))

        e1_all = st.tile([128, JT, ROWS], F32R)    # 128 KB/partition
        smin_all = st.tile([128, JT], F32)         # per-tile col min of s
        biasc_all = st.tile([128, JT], F32)        # +20*smin (exp1 bias)
        sloc_all = st.tile([128, JT], F32)         # local col sums of e1
        colexp = st.tile([128, JT], F32)           # e^{cl - c/2 - lnC/2 + K}
        w2p = st.tile([128, 2, JT], F32R)          # [ones | w2] stats lhsT
        xm2T_sb = st.tile([128, 4, ROWS], F32R)    # -2 * x^T
        xn2_sb = st.tile([2, ROWS], F32R)          # |x|^2 f32r hi/lo rows
        ynp_sb = st.tile([128, JT], F32)           # |y_j|^2, p-major
        dselb = st.tile([128, 32], F32)            # mask row gains
        Kb = st.tile([128, 1], F32)                # K = 10*min(s) broadcast
        stats_sb = st.tile([128, 4, 4], F32)

        # =============== phase 0: stage small inputs =======================
        nc.sync.dma_start(out=xm2T_sb, in_=xm2T.ap())
        nc.sync.dma_start(out=xn2_sb, in_=xn2.ap())
        nc.sync.dma_start(out=ynp_sb, in_=ynp.ap())
        dsel_d = dram.tile([32], F32)
        nc.sync.dma_start(
            out=dsel_d.rearrange("(one r) -> one r", one=1),
            in_=dsel.ap().rearrange("(one r) -> one r", one=1))
        dselb_src = bass.AP(tensor=dsel_d.tensor, offset=dsel_d.offset,
                            ap=[[0, 128], [1, 32]])
        nc.sync.dma_start(out=dselb, in_=dselb_src)

        # =============== phase 1: scores, col stats, e1 ====================
        with tc.tile_pool(name="pyt", bufs=4) as pyt, \
             tc.tile_pool(name="ps_s", bufs=G) as pss, \
             tc.tile_pool(name="pmsk", bufs=2) as pmsk, \
             tc.tile_pool(name="ps_d2", bufs=4, space="PSUM") as ps_d2:
            s_list = {}
            for t in range(JT):
                yt = pyt.tile([128, 4, 128], F32R, tag="yt")
                nc.sync.dma_start(out=yt, in_=ytp.ap()[t])
                d2 = ps_d2.tile([128, ROWS], F32, tag="d2")
                for c in range(4):
                    nc.tensor.matmul(d2[:, :], lhsT=yt[:, c, :],
                                     rhs=xm2T_sb[:, c, :],
                                     start=(c == 0), stop=False)
                nc.tensor.matmul(d2[:, :], lhsT=ones2[:, :], rhs=xn2_sb[:, :],
                                 start=False, stop=True)
                s_t = pss.tile([128, ROWS], F32, tag="s")
                nc.scalar.activation(out=s_t, in_=d2, func=Sqrt,
                                     bias=ynp_sb[:, t:t + 1], scale=1.0)
                if t >= NEG0:
                    m = t - NEG0
                    q = m % 4
                    msk = pmsk.tile([128, 128], F32, tag="m")
                    nc.gpsimd.tensor_scalar_mul(msk, ident, dselb[:, m:m + 1])
                    nc.gpsimd.tensor_tensor(
                        out=s_t[:, q * 128:(q + 1) * 128],
                        in0=s_t[:, q * 128:(q + 1) * 128],
                        in1=msk, op=Alu.add)
                nc.vector.tensor_reduce(out=smin_all[:, t:t + 1], in_=s_t,
                                        op=Alu.min, axis=AX)
                nc.gpsimd.tensor_scalar_mul(biasc_all[:, t:t + 1],
                                            smin_all[:, t:t + 1], -SC)
                s_list[t] = s_t
                if t % G == G - 1:
                    for tt in range(t - G + 1, t + 1):
                        nc.scalar.activation(
                            out=e1_all[:, tt, :], in_=s_list.pop(tt),
                            func=Exp, bias=biasc_all[:, tt:tt + 1],
                            scale=SC, accum_out=sloc_all[:, tt:tt + 1])

        # =============== collective: column stats ==========================
        ag2in = dram.tile([2 * J], F32)
        ag2out = dram.tile([NCORES, 2 * J], F32,
                           addr_space="Local" if _nocc else "Shared")
        nc.sync.dma_start(
            out=ag2in[0:J].rearrange("(p t) -> p t", p=128), in_=smin_all)
        nc.sync.dma_start(
            out=ag2in[J:2 * J].rearrange("(p t) -> p t", p=128), in_=sloc_all)
        if _nocc:
            for r in range(NCORES):
                nc.sync.dma_start(out=ag2out[r, :], in_=ag2in[:])
        else:
            nc.gpsimd.collective_compute(
                "AllGather", Alu.bypass,
                replica_groups=[list(range(NCORES))],
                ins=[ag2in[:]], outs=[ag2out[:, :]])

        # local while AG flies: K = 10 * min(s) broadcast
        sgc = st.tile([128, 1], F32)
        nc.vector.tensor_reduce(out=sgc, in_=smin_all, op=Alu.min, axis=AX)
        tps = ps_small.tile([1, 128], F32, tag="sm")
        nc.tensor.transpose(tps, sgc, ident)
        sg1 = st.tile([1, 1], F32)
        nc.vector.tensor_reduce(out=sg1, in_=tps, op=Alu.min, axis=AX)
        sg2 = st.tile([1, 2], F32R)
        nc.scalar.activation(out=sg2[0:1, 0:1], in_=sg1, func=Copy, scale=10.0)
        nc.scalar.activation(out=sg2[0:1, 1:2], in_=sg1, func=Copy, scale=10.0)
        kb = ps_small.tile([128, 2], F32, tag="sm")
        nc.tensor.matmul(kb[:, :], lhsT=ones_t[:, :], rhs=sg2[:, :],
                         start=True, stop=True)
        nc.vector.tensor_copy(out=Kb, in_=kb[:, 0:1])

        with tc.tile_pool(name="comb", bufs=1) as comb:
            cg = comb.tile([128, NCORES, JT], F32)
            sg_ = comb.tile([128, NCORES, JT], F32)
            for r in range(NCORES):
                nc.sync.dma_start(
                    out=cg[:, r, :],
                    in_=ag2out[r, 0:J].rearrange("(p t) -> p t", p=128))
                nc.sync.dma_start(
                    out=sg_[:, r, :],
                    in_=ag2out[r, J:2 * J].rearrange("(p t) -> p t", p=128))
            gmin = comb.tile([128, JT], F32)
            nc.vector.tensor_reduce(out=gmin,
                                    in_=cg.rearrange("p r t -> p t r"),
                                    op=Alu.min, axis=AX)
            dif = comb.tile([128, JT, NCORES], F32)
            nc.vector.tensor_tensor(
                out=dif, in0=cg.rearrange("p r t -> p t r"),
                in1=gmin[:, :, None].to_broadcast((128, JT, NCORES)),
                op=Alu.subtract)
            nc.scalar.activation(out=dif, in_=dif, func=Exp, scale=SC)
            nc.vector.tensor_tensor(
                out=dif, in0=dif,
                in1=sg_.rearrange("p r t -> p t r"), op=Alu.mult)
            csum = comb.tile([128, JT], F32)
            nc.vector.tensor_reduce(out=csum, in_=dif, op=Alu.add, axis=AX)
            lnc = comb.tile([128, JT], F32)
            nc.scalar.activation(out=lnc, in_=csum, func=Ln)
            # a = 10*gmin - 0.5*lnc ; colexp = exp(-biasc + a + K)
            a1 = comb.tile([128, JT], F32)
            nc.vector.tensor_scalar_mul(a1, gmin, 10.0)
            a2 = comb.tile([128, JT], F32)
            nc.vector.tensor_scalar_mul(a2, lnc, -0.5)
            nc.vector.tensor_tensor(out=a1, in0=a1, in1=a2, op=Alu.add)
            arg = comb.tile([128, JT], F32)
            nc.vector.tensor_tensor(out=arg, in0=a1, in1=biasc_all,
                                    op=Alu.subtract)
            nc.scalar.activation(out=colexp, in_=arg, func=Exp,
                                 bias=Kb[:, 0:1], scale=1.0)
            # w2 = exp(-a + K)
            nc.vector.tensor_scalar_mul(a1, a1, -1.0)
            nc.vector.memset(w2p[:, 0, :].bitcast(F32), 1.0)
            nc.scalar.activation(out=w2p[:, 1, :], in_=a1, func=Exp,
                                 bias=Kb[:, 0:1], scale=1.0)
            if _dbg:
                nc.sync.dma_start(out=dbg_gmin.ap(), in_=gmin)

        if _dbg:
            nc.sync.dma_start(out=dbg_smin.ap(), in_=smin_all)
            nc.sync.dma_start(out=dbg_sloc.ap(), in_=sloc_all)
            nc.sync.dma_start(out=dbg_colexp.ap(), in_=colexp)
            nc.sync.dma_start(out=dbg_w2.ap(), in_=w2p[:, 1, :].bitcast(F32))

        # =============== phase 2: U tiles, V and stats matmuls =============
        with tc.tile_pool(name="p2", bufs=1) as p2s, \
             tc.tile_pool(name="ptp", bufs=4) as ptp, \
             tc.tile_pool(name="py4", bufs=3) as py4, \
             tc.tile_pool(name="pvo", bufs=2) as pvo, \
             tc.tile_pool(name="ps_v", bufs=4, space="PSUM") as ps_v, \
             tc.tile_pool(name="ps_st", bufs=2, space="PSUM") as ps_st:
            vpos_sb = p2s.tile([128, 4, D], F32)
            spos_sb = p2s.tile([2, ROWS], F32)
            sneg_sb = p2s.tile([2, ROWS], F32)
            vps_pos = [ps_v.tile([128, D], F32, name=f"vp{b}", tag="v")
                       for b in range(4)]
            stp_pos = ps_st.tile([2, ROWS], F32, name="sp", tag="s")
            for t in range(JT):
                pos = t < NEG0
                if t == NEG0:
                    vps_neg = [ps_v.tile([128, D], F32, name=f"vn{b}", tag="v")
                               for b in range(4)]
                    stp_neg = ps_st.tile([2, ROWS], F32, name="sn", tag="s")
                vps = vps_pos if pos else vps_neg
                stp = stp_pos if pos else stp_neg
                first, last = t in (0, NEG0), t in (NEG0 - 1, JT - 1)
                tp_t = ptp.tile([128, ROWS], F32R, tag="tp")
                nc.vector.tensor_scalar_mul(tp_t, e1_all[:, t, :],
                                            colexp[:, t:t + 1])
                if t % 4 == 0:
                    ysrc = ypos if pos else yneg
                    g4 = 512 * ((t if pos else t - NEG0) // 4)
                    y4 = py4.tile([128, 4, D], F32R, tag="y4")
                    nc.sync.dma_start(
                        out=y4,
                        in_=ysrc.ap()[g4:g4 + 512, :]
                            .rearrange("(a p) d -> p a d", p=128))
                y_t = y4[:, t % 4, :]
                for b in range(4):
                    nc.tensor.matmul(vps[b][:, :],
                                     lhsT=tp_t[:, b * 128:(b + 1) * 128],
                                     rhs=y_t, start=first, stop=last)
                nc.tensor.matmul(stp[:, :], lhsT=w2p[:, :, t], rhs=tp_t,
                                 start=first, stop=last)
                if _dbg and t == 0:
                    nc.sync.dma_start(out=dbg_tp0.ap(), in_=tp_t.bitcast(F32))
                if t == NEG0 - 1:
                    for b in range(4):
                        nc.vector.tensor_copy(out=vpos_sb[:, b, :],
                                              in_=vps_pos[b])
                    nc.vector.tensor_copy(out=spos_sb, in_=stp_pos)
            nc.vector.tensor_copy(out=sneg_sb, in_=stp_neg)

            # ---- final per-row scales and output ----
            # stats_sb[:, q, :] = [SU_pos, R_pos, SU_neg, R_neg] per i
            for q in range(4):
                for half, src in ((0, spos_sb), (2, sneg_sb)):
                    stq = ps_small.tile([128, 2], F32, tag="sm")
                    nc.tensor.transpose(stq, src[:, q * 128:(q + 1) * 128],
                                        ident[0:2, 0:2])
                    nc.vector.tensor_copy(
                        out=stats_sb[:, q, half:half + 2], in_=stq)
            rtot = st.tile([128, 4], F32)
            nc.vector.tensor_tensor(out=rtot, in0=stats_sb[:, :, 1],
                                    in1=stats_sb[:, :, 3], op=Alu.add)
            rinv = st.tile([128, 4], F32)
            nc.vector.reciprocal(out=rinv, in_=rtot)
            sc_pos = st.tile([128, 4], F32)
            sc_neg = st.tile([128, 4], F32)
            nc.vector.tensor_tensor(out=sc_pos, in0=stats_sb[:, :, 2],
                                    in1=rinv, op=Alu.mult)
            nc.vector.tensor_tensor(out=sc_neg, in0=stats_sb[:, :, 0],
                                    in1=rinv, op=Alu.mult)
            if _dbg:
                nc.sync.dma_start(out=dbg_stat.ap()[0:2, :], in_=spos_sb)
                nc.sync.dma_start(out=dbg_stat.ap()[2:4, :], in_=sneg_sb)
                tpc = p2s.tile([128, ROWS], F32)
                nc.vector.tensor_copy(out=tpc, in_=e1_all[:, 0, :])
                nc.sync.dma_start(out=dbg_e10.ap(), in_=tpc)
            for b in range(4):
                t1 = pvo.tile([128, D], F32, tag="t1")
                nc.scalar.activation(out=t1, in_=vpos_sb[:, b, :], func=Copy,
                                     scale=sc_pos[:, b:b + 1])
                t2 = pvo.tile([128, D], F32, tag="t2")
                nc.vector.tensor_scalar_mul(t2, vps_neg[b], sc_neg[:, b:b + 1])
                vo = pvo.tile([128, D], F32, tag="vo")
                nc.vector.tensor_tensor(out=vo, in0=t1, in1=t2,
                                        op=Alu.subtract)
                nc.sync.dma_start(out=vout.ap()[b * 128:(b + 1) * 128, :],
                                  in_=vo)
    nc.finalize()
    return nc


def _f32r_np(a):
    """Round to f32r (tf32-like, 10 explicit mantissa bits) on host."""
    a = np.asarray(a, np.float32)
    u = a.view(np.uint32)
    u2 = (u + np.uint32(1 << 12)) & np.uint32((0xFFFFFFFF << 13) & 0xFFFFFFFF)
    return u2.view(np.float32)


def _in_maps(x, y_pos, y_neg):
    x = np.asarray(x, np.float32)
    y_pos = np.asarray(y_pos, np.float32)
    y_neg = np.asarray(y_neg, np.float32)
    ycat = np.concatenate([y_pos, y_neg], axis=0)          # (J, D)
    # y^T tiles: ytp[t, d, c, j] = ycat[t*128 + j, c*128 + d]
    ytp = np.ascontiguousarray(
        ycat.reshape(JT, 128, 4, 128).transpose(0, 3, 2, 1))
    yn = np.sum(ycat.astype(np.float64) * ycat, axis=1).astype(np.float32)
    ynp = np.ascontiguousarray(yn.reshape(JT, 128).T)      # (128, JT)
    maps = []
    for k in range(NCORES):
        xs = x[k * ROWS:(k + 1) * ROWS]                    # (ROWS, D)
        xm2T = np.ascontiguousarray(
            (-2.0 * xs.T).reshape(4, 128, ROWS).transpose(1, 0, 2))
        xn = np.sum(xs.astype(np.float64) * xs, axis=1).astype(np.float32)
        xn_hi = _f32r_np(xn)
        xn_lo = _f32r_np(xn - xn_hi)
        ds = np.zeros(32, np.float32)
        ds[4 * k:4 * k + 4] = MASK_VAL
        maps.append({
            "ytp": ytp,
            "ypos": y_pos,
            "yneg": y_neg,
            "xm2T": xm2T,
            "xn2": np.stack([xn_hi, xn_lo]),
            "ynp": ynp,
            "dsel": ds,
        })
    return maps


def _get_runner():
    """Build (once) the jitted 8-core shard_map executable, mirroring
    concourse.bass2jax.run_bass_via_pjrt. Returns a dict with the jit, input
    name order, zero-output templates and output names."""
    if "runner" in _CACHE:
        return _CACHE["runner"]
    import jax
    import jax.numpy as jnp
    from jax.sharding import Mesh, PartitionSpec
    from jax.experimental.shard_map import shard_map
    import concourse.mybir as mybir
    from concourse.bass2jax import (_bass_exec_p, install_neuronx_cc_hook,
                                    partition_id_tensor)

    install_neuronx_cc_hook()
    nc = _build_nc()

    partition_name = (nc.partition_id_tensor.name
                      if nc.partition_id_tensor else None)
    in_names, out_names, out_avals, zero_outs = [], [], [], []
    for alloc in nc.m.functions[0].allocations:
        if not isinstance(alloc, mybir.MemoryLocationSet):
            continue
        if not alloc.memorylocations:
            continue
        name = alloc.memorylocations[0].name
        if alloc.kind == "ExternalInput":
            if name != partition_name:
                in_names.append(name)
        elif alloc.kind == "ExternalOutput":
            shape = tuple(alloc.tensor_shape)
            dtype = mybir.dt.np(alloc.dtype)
            out_names.append(name)
            out_avals.append(jax.core.ShapedArray(shape, dtype))
            zero_outs.append(np.zeros(shape, dtype))
    n_params = len(in_names)
    n_outs = len(out_avals)
    all_in_names = in_names + out_names
    if partition_name is not None:
        all_in_names = all_in_names + [partition_name]
    donate = tuple(range(n_params, n_params + n_outs))

    def _body(*args):
        operands = list(args)
        if partition_name is not None:
            operands.append(partition_id_tensor())
        outs = _bass_exec_p.bind(
            *operands,
            out_avals=tuple(out_avals),
            in_names=tuple(all_in_names),
            out_names=tuple(out_names),
            lowering_input_output_aliases=(),
            sim_require_finite=True,
            sim_require_nnan=True,
            nc=nc,
        )
        return tuple(outs)

    devices = jax.devices()[:NCORES]
    mesh = Mesh(np.asarray(devices), ("core",))
    in_specs = (PartitionSpec("core"),) * (n_params + n_outs)
    out_specs = (PartitionSpec("core"),) * n_outs
    sharded = jax.jit(
        shard_map(_body, mesh=mesh, in_specs=in_specs, out_specs=out_specs,
                  check_rep=False),
        donate_argnums=donate, keep_unused=True)

    runner = dict(sharded=sharded, in_names=in_names, out_names=out_names,
                  out_avals=out_avals, zero_outs=zero_outs, mesh=mesh,
                  n_params=n_params)
    _CACHE["runner"] = runner
    return runner


def _concat_inputs(runner, in_maps):
    return [np.concatenate([np.asarray(in_maps[c][name])
                            for c in range(NCORES)], axis=0)
            for name in runner["in_names"]]


def _concat_zeros(runner):
    return [np.zeros((NCORES * z.shape[0], *z.shape[1:]), z.dtype)
            for z in runner["zero_outs"]]


def _run(in_maps):
    runner = _get_runner()
    out_arrs = runner["sharded"](*_concat_inputs(runner, in_maps),
                                 *_concat_zeros(runner))
    outs = {}
    for i, name in enumerate(runner["out_names"]):
        shp = runner["out_avals"][i].shape
        outs[name] = np.asarray(out_arrs[i]).reshape(NCORES, *shp)
    return outs


def kernel(x, y_pos, y_neg):
    in_dtype = np.asarray(x).dtype
    outs = _run(_in_maps(x, y_pos, y_neg))
    v = outs["vout"].reshape(N, D)
    return np.ascontiguousarray(v).astype(in_dtype, copy=False)


# revision 20
# speedup vs baseline: 1.1856x; 1.1856x over previous
"""Trainium2 Bass kernel for nn_Drifting (dual-softmax contrastive drift).

Computes, for x, y_pos, y_neg all [4096, 512] f32:
    dist_pos = cdist(x, y_pos); dist_neg = cdist(x, y_neg) + eye*1e6
    logit = [-dist_pos, -dist_neg] / 0.05          # [4096, 8192]
    A = sqrt(softmax_row(logit) * softmax_col(logit))
    V = (A_pos * rowsum(A_neg)) @ y_pos - (A_neg * rowsum(A_pos)) @ y_neg

Sharding: rows of x across 8 cores (512 rows each); y replicated. Per-core
layout is TRANSPOSED: scores s[j, i] with j (8192) on partitions (64 tiles
of 128) and local i (512) on the free dim, so column-softmax stats are
free-dim reductions and the output matmul contracts j directly.

Host-side prep (cheap numpy, staged once like the inputs): y^T tiles in
DMA-friendly [t, d, c, j] layout, -2*x^T, |y_j|^2 in [128, 64] p-major
layout, |x_i|^2 split into an f32r hi/lo pair, and a per-core [32]-float
diag-mask row-gain vector (1e6 on the core's own 4 diag tiles).

Pass 1 (flat, 64 j-tiles): d2 via 4 f32r matmuls + a K=2 ones-matmul adding
|x|^2; s = sqrt(d2 + |y|^2[bias]); additive diag mask; column mins; then
e1 = exp(l - cl_j) in groups of 16 (Act sqrt/exp tables cannot coexist; 8
table loads total), with column sums from the activation accumulator.

Math: with K = 10 * min(s_local) (per core) and K0 = -2K, the row-max
factor cancels from the final scale entirely:
  U = e1 * colexp_j,  colexp_j = exp(cl_j - c_j/2 - lnC_j/2 + K)
  V_i = (1/R_i) [SU^neg_i (U@y_pos)_i - SU^pos_i (U@y_neg)_i]
  R_i = sum_j w2_j U[j,i],  w2_j = exp(c_j/2 + lnC_j/2 + K)
so pass 2 per tile is ONE DVE scale (U = e1 * colexp) + 5 f32r matmuls
(4 V-blocks + a fused [ones|w2] stats matmul giving SU and R together).
One small AllGather (64 KB, contiguous 256B runs) shares per-core column
stats (local col mins + e1 col sums); it is the only collective.
"""
import numpy as np

N = 4096
D = 512
NCORES = 8
ROWS = N // NCORES          # 512 local rows (i) per core
J = 2 * N                   # 8192 concat dim
JT = J // 128               # 64 j-tiles
NEG0 = 32                   # first neg tile index
TEMP = 0.05
SC = -1.0 / TEMP            # -20
MASK_VAL = 1e6
G = 16                      # act-table group size (sqrt/exp cannot coexist)

_CACHE = {}


def _build_nc():
    import concourse.bass as bass
    from concourse import bacc
    import concourse.mybir as mybir
    import concourse.tile as tile
    from concourse.masks import make_identity
    from contextlib import ExitStack

    F32 = mybir.dt.float32
    F32R = mybir.dt.float32r
    Exp = mybir.ActivationFunctionType.Exp
    Sqrt = mybir.ActivationFunctionType.Sqrt
    Copy = mybir.ActivationFunctionType.Copy
    Ln = mybir.ActivationFunctionType.Ln
    Alu = mybir.AluOpType
    AX = mybir.AxisListType.X

    nc = bacc.Bacc("TRN2", target_bir_lowering=False, debug=False,
                   num_devices=NCORES)

    ytp = nc.dram_tensor("ytp", [JT, 128, 5, 128], F32R, kind="ExternalInput")
    ypos = nc.dram_tensor("ypos", [N, D], F32R, kind="ExternalInput")
    yneg = nc.dram_tensor("yneg", [N, D], F32R, kind="ExternalInput")
    xm2T = nc.dram_tensor("xm2T", [128, 5, ROWS], F32R, kind="ExternalInput")
    dsel = nc.dram_tensor("dsel", [32], F32, kind="ExternalInput")
    vout = nc.dram_tensor("vout", [ROWS, D], F32, kind="ExternalOutput")
    import os
    _dbg = bool(int(os.environ.get("KERNEL_DEBUG", "0")))
    _nocc = bool(int(os.environ.get("KERNEL_NO_CC", "0")))
    if _dbg:
        dbg_smin = nc.dram_tensor("dbg_smin", [128, JT], F32, kind="ExternalOutput")
        dbg_sloc = nc.dram_tensor("dbg_sloc", [128, JT], F32, kind="ExternalOutput")
        dbg_colexp = nc.dram_tensor("dbg_colexp", [128, JT], F32, kind="ExternalOutput")
        dbg_w2 = nc.dram_tensor("dbg_w2", [128, JT], F32, kind="ExternalOutput")
        dbg_gmin = nc.dram_tensor("dbg_gmin", [128, JT], F32, kind="ExternalOutput")
        dbg_stat = nc.dram_tensor("dbg_stat", [4, ROWS], F32, kind="ExternalOutput")
        dbg_s0 = nc.dram_tensor("dbg_s0", [128, ROWS], F32, kind="ExternalOutput")
        dbg_e10 = nc.dram_tensor("dbg_e10", [128, ROWS], F32, kind="ExternalOutput")
        dbg_tp0 = nc.dram_tensor("dbg_tp0", [128, ROWS], F32, kind="ExternalOutput")

    ones_row_const = nc.inline_tensor(np.ones((1, 128), np.float32), "ones_row_c")

    with tile.TileContext(nc) as tc, ExitStack() as top:
        st = top.enter_context(tc.tile_pool(name="st", bufs=1))
        dram = top.enter_context(tc.tile_pool(name="dram", bufs=1, space="DRAM"))
        ps_small = top.enter_context(
            tc.tile_pool(name="ps_small", bufs=2, space="PSUM"))

        # ---------------- static tiles ----------------
        ident = st.tile([128, 128], F32)
        make_identity(nc, ident)
        ones_t = st.tile([1, 128], F32R)       # K=1 row of ones (lhsT)
        nc.sync.dma_start(out=ones_t, in_=ones_row_const.ap().bitcast(F32R))

        e1_all = st.tile([128, JT, ROWS], F32R)    # 128 KB/partition
        smin_all = st.tile([128, JT], F32)         # per-tile col min of s
        dmin_all = st.tile([128, JT], F32)         # per-tile col min of d2
        biasc_all = st.tile([128, JT], F32)        # +20*smin (exp1 bias)
        sloc_all = st.tile([128, JT], F32)         # local col sums of e1
        colexp = st.tile([128, JT], F32)           # e^{cl - c/2 - lnC/2 + K}
        w2p = st.tile([128, 2, JT], F32R)          # [ones | w2] stats lhsT
        xm2T_sb = st.tile([128, 5, ROWS], F32R)    # -2*x^T | norms rhs chunk
        dselb = st.tile([128, 32], F32)            # mask row gains
        Kb = st.tile([128, 1], F32)                # K = 10*min(s) broadcast
        stats_sb = st.tile([128, 4, 4], F32)

        # =============== phase 0: stage small inputs =======================
        nc.sync.dma_start(out=xm2T_sb, in_=xm2T.ap())
        dsel_d = dram.tile([32], F32)
        nc.sync.dma_start(
            out=dsel_d.rearrange("(one r) -> one r", one=1),
            in_=dsel.ap().rearrange("(one r) -> one r", one=1))
        dselb_src = bass.AP(tensor=dsel_d.tensor, offset=dsel_d.offset,
                            ap=[[0, 128], [1, 32]])
        nc.sync.dma_start(out=dselb, in_=dselb_src)

        # stats collective buffers: one AllGather per half (32 tiles each),
        # the first flies while phase 1 processes the second half.
        HJ = J // 2
        agin = [dram.tile([2 * HJ], F32, name=f"agin{h}") for h in range(2)]
        agout = [dram.tile([NCORES, 2 * HJ], F32, name=f"agout{h}",
                           addr_space="Local" if _nocc else "Shared")
                 for h in range(2)]

        def emit_ag(h):
            hs = slice(h * 32, h * 32 + 32)
            nc.sync.dma_start(
                out=agin[h][0:HJ].rearrange("(p t) -> p t", p=128),
                in_=smin_all[:, hs])
            nc.sync.dma_start(
                out=agin[h][HJ:2 * HJ].rearrange("(p t) -> p t", p=128),
                in_=sloc_all[:, hs])
            if _nocc:
                for r in range(NCORES):
                    nc.sync.dma_start(out=agout[h][r, :], in_=agin[h][:])
            else:
                nc.gpsimd.collective_compute(
                    "AllGather", Alu.bypass,
                    replica_groups=[list(range(NCORES))],
                    ins=[agin[h][:]], outs=[agout[h][:, :]])

        def emit_kb():
            # K = 10 * min(s) over the FIRST HALF only (any per-core constant
            # works: K0 = -2K cancels it from the final scale exactly).
            sgc = st.tile([128, 1], F32)
            nc.vector.tensor_reduce(out=sgc, in_=smin_all[:, 0:32],
                                    op=Alu.min, axis=AX)
            tps = ps_small.tile([1, 128], F32, tag="sm")
            nc.tensor.transpose(tps, sgc, ident)
            sg1 = st.tile([1, 1], F32)
            nc.vector.tensor_reduce(out=sg1, in_=tps, op=Alu.min, axis=AX)
            sg2 = st.tile([1, 2], F32R)
            nc.scalar.activation(out=sg2[0:1, 0:1], in_=sg1, func=Copy,
                                 scale=10.0)
            nc.scalar.activation(out=sg2[0:1, 1:2], in_=sg1, func=Copy,
                                 scale=10.0)
            kb = ps_small.tile([128, 2], F32, tag="sm")
            nc.tensor.matmul(kb[:, :], lhsT=ones_t[:, :], rhs=sg2[:, :],
                             start=True, stop=True)
            nc.vector.tensor_copy(out=Kb, in_=kb[:, 0:1])

        def emit_combine(h, comb):
            hs = slice(h * 32, h * 32 + 32)
            cg = comb.tile([128, NCORES, 32], F32, tag="cg")
            sg_ = comb.tile([128, NCORES, 32], F32, tag="sg")
            for r in range(NCORES):
                nc.sync.dma_start(
                    out=cg[:, r, :],
                    in_=agout[h][r, 0:HJ].rearrange("(p t) -> p t", p=128))
                nc.sync.dma_start(
                    out=sg_[:, r, :],
                    in_=agout[h][r, HJ:2 * HJ].rearrange("(p t) -> p t",
                                                         p=128))
            gmin = comb.tile([128, 32], F32, tag="gm")
            nc.vector.tensor_reduce(out=gmin,
                                    in_=cg.rearrange("p r t -> p t r"),
                                    op=Alu.min, axis=AX)
            dif = comb.tile([128, 32, NCORES], F32, tag="df")
            nc.vector.tensor_tensor(
                out=dif, in0=cg.rearrange("p r t -> p t r"),
                in1=gmin[:, :, None].to_broadcast((128, 32, NCORES)),
                op=Alu.subtract)
            nc.scalar.activation(out=dif, in_=dif, func=Exp, scale=SC)
            nc.vector.tensor_tensor(
                out=dif, in0=dif,
                in1=sg_.rearrange("p r t -> p t r"), op=Alu.mult)
            csum = comb.tile([128, 32], F32, tag="cs")
            nc.vector.tensor_reduce(out=csum, in_=dif, op=Alu.add, axis=AX)
            lnc = comb.tile([128, 32], F32, tag="ln")
            nc.scalar.activation(out=lnc, in_=csum, func=Ln)
            # a = 10*gmin - 0.5*lnc ; colexp = exp(-biasc + a + K)
            a1 = comb.tile([128, 32], F32, tag="a1")
            nc.vector.tensor_scalar_mul(a1, gmin, 10.0)
            a2 = comb.tile([128, 32], F32, tag="a2")
            nc.vector.tensor_scalar_mul(a2, lnc, -0.5)
            nc.vector.tensor_tensor(out=a1, in0=a1, in1=a2, op=Alu.add)
            arg = comb.tile([128, 32], F32, tag="ar")
            nc.vector.tensor_tensor(out=arg, in0=a1, in1=biasc_all[:, hs],
                                    op=Alu.subtract)
            nc.scalar.activation(out=colexp[:, hs], in_=arg, func=Exp,
                                 bias=Kb[:, 0:1], scale=1.0)
            # w2 = exp(-a + K)
            nc.vector.tensor_scalar_mul(a1, a1, -1.0)
            nc.scalar.activation(out=w2p[:, 1, hs], in_=a1, func=Exp,
                                 bias=Kb[:, 0:1], scale=1.0)

        nc.vector.memset(w2p[:, 0, :].bitcast(F32), 1.0)

        # =============== phase 1: scores, col stats, e1 ====================
        comb_pool = top.enter_context(tc.tile_pool(name="comb", bufs=2))
        with tc.tile_pool(name="pyt", bufs=4) as pyt, \
             tc.tile_pool(name="ps_s", bufs=G) as pss, \
             tc.tile_pool(name="pmsk", bufs=2) as pmsk, \
             tc.tile_pool(name="ps_d2", bufs=3, space="PSUM") as ps_d2:
            s_list = {}
            for t in range(JT):
                yt = pyt.tile([128, 5, 128], F32R, tag="yt")
                nc.sync.dma_start(out=yt, in_=ytp.ap()[t])
                d2 = ps_d2.tile([128, ROWS], F32, tag="d2")
                for c in range(4):
                    nc.tensor.matmul(d2[:, :], lhsT=yt[:, c, :],
                                     rhs=xm2T_sb[:, c, :],
                                     start=(c == 0), stop=False)
                nc.tensor.matmul(d2[:, :], lhsT=yt[0:4, 4, :],
                                 rhs=xm2T_sb[0:4, 4, :],
                                 start=False, stop=True)
                s_t = pss.tile([128, ROWS], F32, tag="s")
                nc.scalar.activation(out=s_t, in_=d2, func=Sqrt)
                nc.vector.tensor_reduce(out=dmin_all[:, t:t + 1], in_=d2,
                                        op=Alu.min, axis=AX)
                if t >= NEG0:
                    m = t - NEG0
                    q = m % 4
                    msk = pmsk.tile([128, 128], F32, tag="m")
                    nc.vector.tensor_scalar_mul(msk, ident, dselb[:, m:m + 1])
                    nc.vector.tensor_tensor(
                        out=s_t[:, q * 128:(q + 1) * 128],
                        in0=s_t[:, q * 128:(q + 1) * 128],
                        in1=msk, op=Alu.add)
                s_list[t] = s_t
                if t % G == G - 1:
                    g0 = t - G + 1
                    nc.scalar.activation(out=smin_all[:, g0:t + 1],
                                         in_=dmin_all[:, g0:t + 1], func=Sqrt)
                    nc.gpsimd.tensor_scalar_mul(biasc_all[:, g0:t + 1],
                                                smin_all[:, g0:t + 1], -SC)
                    for tt in range(g0, t + 1):
                        nc.scalar.activation(
                            out=e1_all[:, tt, :], in_=s_list.pop(tt),
                            func=Exp, bias=biasc_all[:, tt:tt + 1], scale=SC)
                        nc.gpsimd.tensor_reduce(
                            out=sloc_all[:, tt:tt + 1],
                            in_=e1_all[:, tt, :].bitcast(F32),
                            op=Alu.add, axis=AX)
                    if t == 31:
                        emit_ag(0)
                        emit_kb()
                        emit_combine(0, comb_pool)

        emit_ag(1)
        emit_combine(1, comb_pool)
        if _dbg:
            nc.sync.dma_start(out=dbg_gmin.ap(), in_=dmin_all)

        if _dbg:
            nc.sync.dma_start(out=dbg_smin.ap(), in_=smin_all)
            nc.sync.dma_start(out=dbg_sloc.ap(), in_=sloc_all)
            nc.sync.dma_start(out=dbg_colexp.ap(), in_=colexp)
            nc.sync.dma_start(out=dbg_w2.ap(), in_=w2p[:, 1, :].bitcast(F32))

        # =============== phase 2: U tiles, V and stats matmuls =============
        with tc.tile_pool(name="p2", bufs=1) as p2s, \
             tc.tile_pool(name="ptp", bufs=4) as ptp, \
             tc.tile_pool(name="py4", bufs=3) as py4, \
             tc.tile_pool(name="pvo", bufs=2) as pvo, \
             tc.tile_pool(name="ps_v", bufs=4, space="PSUM") as ps_v, \
             tc.tile_pool(name="ps_st", bufs=2, space="PSUM") as ps_st:
            vpos_sb = p2s.tile([128, 4, D], F32)
            spos_sb = p2s.tile([2, ROWS], F32)
            sneg_sb = p2s.tile([2, ROWS], F32)
            vps_pos = [ps_v.tile([128, D], F32, name=f"vp{b}", tag="v")
                       for b in range(4)]
            stp_pos = ps_st.tile([2, ROWS], F32, name="sp", tag="s")
            for t in range(JT):
                pos = t < NEG0
                if t == NEG0:
                    vps_neg = [ps_v.tile([128, D], F32, name=f"vn{b}", tag="v")
                               for b in range(4)]
                    stp_neg = ps_st.tile([2, ROWS], F32, name="sn", tag="s")
                vps = vps_pos if pos else vps_neg
                stp = stp_pos if pos else stp_neg
                first, last = t in (0, NEG0), t in (NEG0 - 1, JT - 1)
                tp_t = ptp.tile([128, ROWS], F32R, tag="tp")
                nc.vector.tensor_scalar_mul(tp_t, e1_all[:, t, :],
                                            colexp[:, t:t + 1])
                if t % 4 == 0:
                    ysrc = ypos if pos else yneg
                    g4 = 512 * ((t if pos else t - NEG0) // 4)
                    y4 = py4.tile([128, 4, D], F32R, tag="y4")
                    nc.sync.dma_start(
                        out=y4,
                        in_=ysrc.ap()[g4:g4 + 512, :]
                            .rearrange("(a p) d -> p a d", p=128))
                y_t = y4[:, t % 4, :]
                for b in range(4):
                    nc.tensor.matmul(vps[b][:, :],
                                     lhsT=tp_t[:, b * 128:(b + 1) * 128],
                                     rhs=y_t, start=first, stop=last)
                nc.tensor.matmul(stp[:, :], lhsT=w2p[:, :, t], rhs=tp_t,
                                 start=first, stop=last)
                if _dbg and t == 0:
                    nc.sync.dma_start(out=dbg_tp0.ap(), in_=tp_t.bitcast(F32))
                if t == NEG0 - 1:
                    for b in range(4):
                        nc.vector.tensor_copy(out=vpos_sb[:, b, :],
                                              in_=vps_pos[b])
                    nc.vector.tensor_copy(out=spos_sb, in_=stp_pos)
            nc.vector.tensor_copy(out=sneg_sb, in_=stp_neg)

            # ---- final per-row scales and output ----
            # stats_sb[:, q, :] = [SU_pos, R_pos, SU_neg, R_neg] per i
            for q in range(4):
                for half, src in ((0, spos_sb), (2, sneg_sb)):
                    stq = ps_small.tile([128, 2], F32, tag="sm")
                    nc.tensor.transpose(stq, src[:, q * 128:(q + 1) * 128],
                                        ident[0:2, 0:2])
                    nc.vector.tensor_copy(
                        out=stats_sb[:, q, half:half + 2], in_=stq)
            rtot = st.tile([128, 4], F32)
            nc.vector.tensor_tensor(out=rtot, in0=stats_sb[:, :, 1],
                                    in1=stats_sb[:, :, 3], op=Alu.add)
            rinv = st.tile([128, 4], F32)
            nc.vector.reciprocal(out=rinv, in_=rtot)
            sc_pos = st.tile([128, 4], F32)
            sc_neg = st.tile([128, 4], F32)
            nc.vector.tensor_tensor(out=sc_pos, in0=stats_sb[:, :, 2],
                                    in1=rinv, op=Alu.mult)
            nc.vector.tensor_tensor(out=sc_neg, in0=stats_sb[:, :, 0],
                                    in1=rinv, op=Alu.mult)
            if _dbg:
                nc.sync.dma_start(out=dbg_stat.ap()[0:2, :], in_=spos_sb)
                nc.sync.dma_start(out=dbg_stat.ap()[2:4, :], in_=sneg_sb)
                tpc = p2s.tile([128, ROWS], F32)
                nc.vector.tensor_copy(out=tpc, in_=e1_all[:, 0, :])
                nc.sync.dma_start(out=dbg_e10.ap(), in_=tpc)
            for b in range(4):
                t1 = pvo.tile([128, D], F32, tag="t1")
                nc.scalar.activation(out=t1, in_=vpos_sb[:, b, :], func=Copy,
                                     scale=sc_pos[:, b:b + 1])
                t2 = pvo.tile([128, D], F32, tag="t2")
                nc.vector.tensor_scalar_mul(t2, vps_neg[b], sc_neg[:, b:b + 1])
                vo = pvo.tile([128, D], F32, tag="vo")
                nc.gpsimd.tensor_tensor(out=vo, in0=t1, in1=t2,
                                        op=Alu.subtract)
                nc.sync.dma_start(out=vout.ap()[b * 128:(b + 1) * 128, :],
                                  in_=vo)
    nc.finalize()
    return nc


def _f32r_np(a):
    """Round to f32r (tf32-like, 10 explicit mantissa bits) on host."""
    a = np.asarray(a, np.float32)
    u = a.view(np.uint32)
    u2 = (u + np.uint32(1 << 12)) & np.uint32((0xFFFFFFFF << 13) & 0xFFFFFFFF)
    return u2.view(np.float32)


def _in_maps(x, y_pos, y_neg):
    x = np.asarray(x, np.float32)
    y_pos = np.asarray(y_pos, np.float32)
    y_neg = np.asarray(y_neg, np.float32)
    ycat = np.concatenate([y_pos, y_neg], axis=0)          # (J, D)
    # y^T tiles (chunks 0-3): ytp[t, d, c, j] = ycat[t*128 + j, c*128 + d].
    # Chunk 4 (K=4) adds the norms: d2 += 1*xn_hi + 1*xn_lo + yn_hi*1 + yn_lo*1
    ytp = np.zeros((JT, 128, 5, 128), np.float32)
    ytp[:, :, 0:4, :] = ycat.reshape(JT, 128, 4, 128).transpose(0, 3, 2, 1)
    yn = np.sum(ycat.astype(np.float64) * ycat, axis=1).astype(np.float32)
    yn_hi = _f32r_np(yn)
    yn_lo = _f32r_np(yn - yn_hi)
    ytp[:, 0, 4, :] = 1.0
    ytp[:, 1, 4, :] = 1.0
    ytp[:, 2, 4, :] = yn_hi.reshape(JT, 128)
    ytp[:, 3, 4, :] = yn_lo.reshape(JT, 128)
    maps = []
    for k in range(NCORES):
        xs = x[k * ROWS:(k + 1) * ROWS]                    # (ROWS, D)
        xm2T = np.zeros((128, 5, ROWS), np.float32)
        xm2T[:, 0:4, :] = (-2.0 * xs.T).reshape(4, 128, ROWS).transpose(1, 0, 2)
        xn = np.sum(xs.astype(np.float64) * xs, axis=1).astype(np.float32)
        xm2T[0, 4, :] = _f32r_np(xn)
        xm2T[1, 4, :] = _f32r_np(xn - xm2T[0, 4, :])
        xm2T[2, 4, :] = 1.0
        xm2T[3, 4, :] = 1.0
        ds = np.zeros(32, np.float32)
        ds[4 * k:4 * k + 4] = MASK_VAL
        maps.append({
            "ytp": ytp,
            "ypos": y_pos,
            "yneg": y_neg,
            "xm2T": xm2T,
            "dsel": ds,
        })
    return maps


def _get_runner():
    """Build (once) the jitted 8-core shard_map executable, mirroring
    concourse.bass2jax.run_bass_via_pjrt. Returns a dict with the jit, input
    name order, zero-output templates and output names."""
    if "runner" in _CACHE:
        return _CACHE["runner"]
    import jax
    import jax.numpy as jnp
    from jax.sharding import Mesh, PartitionSpec
    from jax.experimental.shard_map import shard_map
    import concourse.mybir as mybir
    from concourse.bass2jax import (_bass_exec_p, install_neuronx_cc_hook,
                                    partition_id_tensor)

    install_neuronx_cc_hook()
    nc = _build_nc()

    partition_name = (nc.partition_id_tensor.name
                      if nc.partition_id_tensor else None)
    in_names, out_names, out_avals, zero_outs = [], [], [], []
    for alloc in nc.m.functions[0].allocations:
        if not isinstance(alloc, mybir.MemoryLocationSet):
            continue
        if not alloc.memorylocations:
            continue
        name = alloc.memorylocations[0].name
        if alloc.kind == "ExternalInput":
            if name != partition_name:
                in_names.append(name)
        elif alloc.kind == "ExternalOutput":
            shape = tuple(alloc.tensor_shape)
            dtype = mybir.dt.np(alloc.dtype)
            out_names.append(name)
            out_avals.append(jax.core.ShapedArray(shape, dtype))
            zero_outs.append(np.zeros(shape, dtype))
    n_params = len(in_names)
    n_outs = len(out_avals)
    all_in_names = in_names + out_names
    if partition_name is not None:
        all_in_names = all_in_names + [partition_name]
    donate = tuple(range(n_params, n_params + n_outs))

    def _body(*args):
        operands = list(args)
        if partition_name is not None:
            operands.append(partition_id_tensor())
        outs = _bass_exec_p.bind(
            *operands,
            out_avals=tuple(out_avals),
            in_names=tuple(all_in_names),
            out_names=tuple(out_names),
            lowering_input_output_aliases=(),
            sim_require_finite=True,
            sim_require_nnan=True,
            nc=nc,
        )
        return tuple(outs)

    devices = jax.devices()[:NCORES]
    mesh = Mesh(np.asarray(devices), ("core",))
    in_specs = (PartitionSpec("core"),) * (n_params + n_outs)
    out_specs = (PartitionSpec("core"),) * n_outs
    sharded = jax.jit(
        shard_map(_body, mesh=mesh, in_specs=in_specs, out_specs=out_specs,
                  check_rep=False),
        donate_argnums=donate, keep_unused=True)

    runner = dict(sharded=sharded, in_names=in_names, out_names=out_names,
                  out_avals=out_avals, zero_outs=zero_outs, mesh=mesh,
                  n_params=n_params)
    _CACHE["runner"] = runner
    return runner


def _concat_inputs(runner, in_maps):
    return [np.concatenate([np.asarray(in_maps[c][name])
                            for c in range(NCORES)], axis=0)
            for name in runner["in_names"]]


def _concat_zeros(runner):
    return [np.zeros((NCORES * z.shape[0], *z.shape[1:]), z.dtype)
            for z in runner["zero_outs"]]


def _run(in_maps):
    runner = _get_runner()
    out_arrs = runner["sharded"](*_concat_inputs(runner, in_maps),
                                 *_concat_zeros(runner))
    outs = {}
    for i, name in enumerate(runner["out_names"]):
        shp = runner["out_avals"][i].shape
        outs[name] = np.asarray(out_arrs[i]).reshape(NCORES, *shp)
    return outs


def kernel(x, y_pos, y_neg):
    in_dtype = np.asarray(x).dtype
    outs = _run(_in_maps(x, y_pos, y_neg))
    v = outs["vout"].reshape(N, D)
    return np.ascontiguousarray(v).astype(in_dtype, copy=False)
